# revision 1
# baseline (speedup 1.0000x reference)
"""Trainium2 Bass kernel for additive-attention pooling.

reference math:
    scores[b,t] = tanh(q[b]) @ vw_a + tanh(c[b,t]) @ vw_b
    attn        = softmax(where(mask<1, -1e10, scores), axis=t)
    out[b,e]    = sum_t attn[b,t] * c[b,t,e]

Softmax is shift-invariant and the query term is constant over t, so the
output does not depend on `query` or `v_w[:E]` at all.  Per batch row,
in a single pass over context:
    s_t  = sum_e (tanh(c_te) + mbias_t) * w2_e
         = tanh(c_t).w2 + (mask_t-1)*1e9     (DVE affine_mul_reduce, f32;
                                              per-partition bias (m-1)*1e9/S,
                                              S = sum(w2), pushes masked rows
                                              to score ~ -1e9)
    p_t  = exp(s_t)                          (ACT, bf16 out; masked -> 0)
    out  = (sum_t p_t*c_t) / (sum_t p_t)     (PE bf16 matmuls, f32 PSUM)

Engine placement (measured): f32 matmul runs 2-pass LOW_HIGH on PE (~4x
bf16 cost), and every engine's explicit f32->bf16 cast is too slow
(GPSIMD 3.7us, DVE/ACT ~1.1us per tile) — so the matmul rhs is a ZERO
COST bf16 view of the f32 tile: bitcast to bf16 and read the odd
(high-half) 2-byte lanes with stride 2.  That is exactly bf16
truncation of each f32 (~1ulp, fine for the 2e-2 gate).  A f32 1.0
ones-column embedded in each tile (bf16-view 1.0 exactly) makes the
same matmul accumulate the softmax denominator.

t-tiles pack 2 context rows per partition ([128 x (j=2, 769)]) so each
partition streams ~6KB from HBM per tile and tanh batches to one ACT
op per 256 rows.  w2 (replicated to 128 partitions) and the mask-bias
scale R = 1e9/sum(w2) are prepared host-side — they are tiny and would
otherwise serialize ~10us of on-device setup before the first score op.

Sharding: pure data parallel, batch 16 -> 2 per core on 8 cores; w2
replicated.  No collectives needed.
"""

import sys

for _p in ("/opt/trn_rl_repo", "/root/.axon_site/_ro/trn_rl_repo"):
    if _p not in sys.path:
        sys.path.append(_p)

import numpy as np

B, T, E = 16, 4096, 768
NCORES = 8
BPC = B // NCORES  # batches per core
P = 128            # partitions per tile
J = 2              # context rows per partition
G = T // (P * J)   # 16 t-tiles per batch
NEG_BIG = 1.0e9    # exp(-1e9) == 0.0
EB = E + 1         # tile row: 768 data + 1 ones column

_cache = {}


def _build_program():
    import concourse.tile as tile
    from concourse import bacc, mybir

    f32 = mybir.dt.float32
    bf16 = mybir.dt.bfloat16
    i32 = mybir.dt.int32
    AF = mybir.ActivationFunctionType
    ALU = mybir.AluOpType

    nc = bacc.Bacc(
        "TRN2",
        target_bir_lowering=False,
        debug=False,
        enable_asserts=False,
        num_devices=NCORES,
    )
    ctx_d = nc.dram_tensor("ctx", [BPC, T, E], f32, kind="ExternalInput")
    mask_d = nc.dram_tensor("mask", [BPC, T], i32, kind="ExternalInput")
    w2_d = nc.dram_tensor("w2rep", [P, E], f32, kind="ExternalInput")
    r_d = nc.dram_tensor("rrep", [P, 1], f32, kind="ExternalInput")
    out_d = nc.dram_tensor("out", [BPC, EB], f32, kind="ExternalOutput")

    with tile.TileContext(nc) as tc:
        with (
            tc.tile_pool(name="const", bufs=1) as const_pool,
            tc.tile_pool(name="cin", bufs=6) as c_pool,
            tc.tile_pool(name="tanh", bufs=4) as t_pool,
            tc.tile_pool(name="small", bufs=8) as s_pool,
            tc.tile_pool(name="batch", bufs=2) as b_pool,
            tc.tile_pool(name="paccum", bufs=2, space="PSUM") as pa_pool,
        ):
            def load_tile(b, g, split=1):
                c = c_pool.tile([P, J * EB], f32)
                c3 = c[:].rearrange("p (j e) -> p j e", j=J)
                pq = P // split
                for q in range(split):
                    t0 = g * P * J + q * pq * J
                    nc.sync.dma_start(
                        c3[q * pq:(q + 1) * pq, :, 0:E],
                        ctx_d[b, t0:t0 + pq * J, :].rearrange(
                            "(p j) e -> p j e", j=J
                        ),
                    )
                # ones columns at the end of each j slice (f32 1.0 is
                # exactly 1.0 in the truncated-bf16 view); GPSIMD is idle
                nc.gpsimd.memset(c3[:, :, E:EB], 1.0)
                return c

            # first context tiles ahead of the setup DMAs: each dma_start
            # costs ~620ns of serial trigger-issue on the sync engine, so
            # tile 0 must be first in line for compute to ramp early
            preloaded = {g: load_tile(0, g) for g in range(2)}

            # ---- constants (prepared host-side, one DMA each) ----
            w2_rep = const_pool.tile([P, E], f32)
            nc.sync.dma_start(w2_rep[:], w2_d[:])
            r_rep = const_pool.tile([P, 1], f32)
            nc.sync.dma_start(r_rep[:], r_d[:])

            for b in range(BPC):
                # mask -> per-(p, g*J+j) amr bias: 0 kept, -R masked
                mask_i = b_pool.tile([P, G * J], i32)
                nc.sync.dma_start(
                    mask_i[:].rearrange("p (g j) -> p g j", g=G, j=J),
                    mask_d[b].rearrange("(g p j) -> p g j", p=P, j=J),
                )
                mask_f = b_pool.tile([P, G * J], f32)
                nc.vector.tensor_copy(mask_f[:], mask_i[:])
                mbias = b_pool.tile([P, G * J], f32)
                nc.vector.tensor_scalar(
                    mbias[:], mask_f[:], r_rep[:], r_rep[:],
                    op0=ALU.mult, op1=ALU.subtract,
                )

                acc = pa_pool.tile([1, EB], f32)  # [sum p*c | sum p]

                for g in range(G):
                    c = preloaded.pop(g, None) if b == 0 else None
                    if c is None:
                        c = load_tile(b, g)
                    # zero-cost truncated-bf16 view: odd u16 lane of each f32
                    c_hi = c[:].bitcast(bf16).rearrange(
                        "p (n two) -> p n two", two=2
                    )[:, :, 1]

                    th = t_pool.tile([P, J * E], f32)
                    nc.scalar.activation(
                        th[:].rearrange("p (j e) -> p j e", j=J),
                        c[:].rearrange("p (j e) -> p j e", j=J)[:, :, 0:E],
                        AF.Tanh,
                    )

                    s2 = s_pool.tile([P, J], f32)
                    for j in range(J):
                        sl = slice(j * E, (j + 1) * E)
                        nc.vector.affine_mul_reduce(
                            th[:, sl], s2[:, j:j + 1], th[:, sl], w2_rep[:],
                            1.0, mbias[:, g * J + j:g * J + j + 1],
                        )

                    p2 = s_pool.tile([P, J], bf16)
                    nc.scalar.activation(p2[:], s2[:], AF.Exp)

                    first, last = g == 0, g == G - 1
                    for j in range(J):
                        lhsT = p2[:, j:j + 1]
                        st = first and j == 0
                        sp = last and j == J - 1
                        nc.tensor.matmul(
                            acc[:, 0:512], lhsT=lhsT,
                            rhs=c_hi[:, j * EB:j * EB + 512], start=st, stop=sp,
                        )
                        nc.tensor.matmul(
                            acc[:, 512:EB], lhsT=lhsT,
                            rhs=c_hi[:, j * EB + 512:(j + 1) * EB],
                            start=st, stop=sp,
                        )

                # copy [num | den] out; the divide happens host-side (16x768
                # divides) which drops ~1us of serial tail per batch
                out_sb = s_pool.tile([1, EB], f32)
                nc.vector.tensor_copy(out_sb[:], acc[:])
                nc.sync.dma_start(out_d[b:b + 1, :], out_sb[:])

    nc.compile()
    return nc


def _get_program():
    if "nc" not in _cache:
        _cache["nc"] = _build_program()
    return _cache["nc"]


def kernel(query, context, mask, v_w):
    import time
    from concourse.bass_utils import run_bass_kernel_spmd

    nc = _get_program()
    w2 = np.asarray(v_w[E:], dtype=np.float32)
    w2_rep = np.ascontiguousarray(np.broadcast_to(w2, (P, E)))
    r = np.float32(NEG_BIG) / w2.sum(dtype=np.float32)
    r_rep = np.full((P, 1), r, dtype=np.float32)
    in_maps = [
        {
            "ctx": np.ascontiguousarray(context[i * BPC:(i + 1) * BPC]),
            "mask": np.ascontiguousarray(mask[i * BPC:(i + 1) * BPC]),
            "w2rep": w2_rep,
            "rrep": r_rep,
        }
        for i in range(NCORES)
    ]
    last_err = None
    for attempt in range(3):
        try:
            res = run_bass_kernel_spmd(nc, in_maps, list(range(NCORES)))
            raw = np.concatenate(
                [res.results[i]["out"] for i in range(NCORES)], axis=0
            )
            return raw[:, :E] / raw[:, E:EB]
        except Exception as e:  # transient axon/device hiccups
            last_err = e
            time.sleep(5)
    raise last_err



# revision 7
# speedup vs baseline: 1.7707x; 1.7707x over previous
"""Trainium2 Bass kernel for additive-attention pooling.

reference math:
    scores[b,t] = tanh(q[b]) @ vw_a + tanh(c[b,t]) @ vw_b
    attn        = softmax(where(mask<1, -1e10, scores), axis=t)
    out[b,e]    = sum_t attn[b,t] * c[b,t,e]

Softmax is shift-invariant and the query term is constant over t, so the
output depends only on `context`, `mask` and v_w[E:] (=: w2).  Masked
rows get weight exactly 0, and the mask is ~50% zeros — so the host
compacts each batch to its unmasked rows (zero-padded up to a multiple
of 384) and casts to bf16.  That cuts HBM traffic 4x vs the f32 full-T
stream, which moves the kernel from DMA-bound to ACT(tanh)-bound.

Device program, per batch row, streaming over 384-row tiles
[128 partitions x (j=3 rows) x 768] bf16:
    th   = tanh(c)                       ACT, one op per tile
    s_j  = sum_e th*w2                   DVE tensor_tensor_reduce (bf16 2x)
    p    = exp(s)                        ACT, tiny
    num += p^T @ c                       PE bf16 matmuls into f32 PSUM
    p -> HBM                             denominator summed host-side
Pad rows cost tanh(0)=0 -> s=0 -> p=1, contribute 0 to num (c row is
zero) and are excluded from the host-side denominator sum, so no mask
logic exists on device at all.

Sharding: pure data parallel, batch 16 -> 2 per core on 8 cores; w2
replicated (host-broadcast to 128 partitions).  No collectives.
"""

import sys

for _p in ("/opt/trn_rl_repo", "/root/.axon_site/_ro/trn_rl_repo"):
    if _p not in sys.path:
        sys.path.append(_p)

import numpy as np
import ml_dtypes

B, T, E = 16, 4096, 768
NCORES = 8
BPC = B // NCORES  # batches per core
P = 128            # partitions per tile
J = 3              # context rows per partition
RPT = P * J        # rows per tile = 384

_cache = {}


def _build_program(Gp):
    import concourse.tile as tile
    from concourse import bacc, mybir

    f32 = mybir.dt.float32
    bf16 = mybir.dt.bfloat16
    AF = mybir.ActivationFunctionType
    ALU = mybir.AluOpType
    S = Gp * J  # score columns per batch

    nc = bacc.Bacc(
        "TRN2",
        target_bir_lowering=False,
        debug=False,
        enable_asserts=False,
        num_devices=NCORES,
    )
    ctx_d = nc.dram_tensor("ctx", [BPC, Gp * RPT, E], bf16, kind="ExternalInput")
    w2_d = nc.dram_tensor("w2rep", [P, E], bf16, kind="ExternalInput")
    num_d = nc.dram_tensor("num", [BPC, E], f32, kind="ExternalOutput")
    p_d = nc.dram_tensor("pout", [BPC, P, S], bf16, kind="ExternalOutput")

    with tile.TileContext(nc) as tc:
        with (
            tc.tile_pool(name="const", bufs=1) as const_pool,
            tc.tile_pool(name="cin", bufs=6) as c_pool,
            tc.tile_pool(name="tanh", bufs=3) as t_pool,
            tc.tile_pool(name="sb", bufs=2) as sb_pool,
            tc.tile_pool(name="small", bufs=2) as s_pool,
            tc.tile_pool(name="paccum", bufs=2, space="PSUM") as pa_pool,
        ):
            def load_tile(b, g):
                # one contiguous 4608B run per partition on both sides
                c = c_pool.tile([P, J * E], bf16)
                nc.sync.dma_start(
                    c[:].rearrange("p (j e) -> p j e", j=J),
                    ctx_d[b, g * RPT:(g + 1) * RPT, :].rearrange(
                        "(p j) e -> p j e", j=J
                    ),
                )
                return c

            # tile 0 leads on the SP queue (ACT is the bottleneck engine,
            # its first tanh should start ASAP); w2 right behind so the
            # first score op isn't starved
            preloaded = {g: load_tile(0, g) for g in range(min(2, Gp))}
            w2_rep = const_pool.tile([P, E], bf16)
            nc.sync.dma_start(w2_rep[:], w2_d[:])
            if Gp > 2:
                preloaded[2] = load_tile(0, 2)

            for b in range(BPC):
                sbuf = sb_pool.tile([P, S], f32)
                pbuf = sb_pool.tile([P, S], bf16)
                acc = pa_pool.tile([1, E], f32)
                for g in range(Gp):
                    c = preloaded.pop(g, None) if b == 0 else None
                    if c is None:
                        c = load_tile(b, g)
                    th = t_pool.tile([P, J * E], bf16)
                    nc.scalar.activation(th[:], c[:], AF.Tanh)

                    last = g == Gp - 1
                    # on the last tile exp per j so the final matmuls
                    # don't wait on the whole tile's score chain
                    jgrp = [[j] for j in range(J)] if last else [list(range(J))]
                    for grp in jgrp:
                        for j in grp:
                            sl = slice(j * E, (j + 1) * E)
                            # tensor_tensor_reduce wedges the exec unit on
                            # this runtime build; amr is HW-proven
                            nc.vector.affine_mul_reduce(
                                th[:, sl],
                                sbuf[:, g * J + j:g * J + j + 1],
                                th[:, sl], w2_rep[:], 1.0, 0.0,
                            )
                        c0, c1 = g * J + grp[0], g * J + grp[-1] + 1
                        nc.scalar.activation(
                            pbuf[:, c0:c1], sbuf[:, c0:c1], AF.Exp
                        )
                        for j in grp:
                            lhsT = pbuf[:, g * J + j:g * J + j + 1]
                            st = g == 0 and j == 0
                            sp = last and j == J - 1
                            nc.tensor.matmul(
                                acc[:, 0:512], lhsT=lhsT,
                                rhs=c[:, j * E:j * E + 512],
                                start=st, stop=sp,
                            )
                            nc.tensor.matmul(
                                acc[:, 512:E], lhsT=lhsT,
                                rhs=c[:, j * E + 512:(j + 1) * E],
                                start=st, stop=sp,
                            )
                # denominator values + numerator out
                nc.sync.dma_start(p_d[b], pbuf[:])
                osb = s_pool.tile([1, E], f32)
                nc.vector.tensor_copy(osb[:], acc[:])
                nc.sync.dma_start(num_d[b:b + 1, :], osb[:])

    nc.compile()
    return nc


def _get_program(Gp):
    key = ("nc", Gp)
    if key not in _cache:
        _cache[key] = _build_program(Gp)
    return _cache[key]


def _prepare(context, mask, v_w):
    """Compact unmasked rows per batch, pad to a tile multiple, bf16."""
    bf16 = ml_dtypes.bfloat16
    m = np.asarray(mask)
    counts = m.sum(axis=1).astype(np.int64)
    Tp = int(max(RPT, -(-counts.max() // RPT) * RPT))
    Gp = Tp // RPT

    ctx = np.asarray(context, dtype=np.float32)
    ctx_c = np.zeros((B, Tp, E), dtype=bf16)
    for b in range(B):
        idx = np.flatnonzero(m[b])
        ctx_c[b, :len(idx)] = ctx[b, idx].astype(bf16)

    w2 = np.asarray(v_w[E:], dtype=np.float32).astype(bf16)
    w2_rep = np.ascontiguousarray(np.broadcast_to(w2, (P, E)))
    in_maps = [
        {
            "ctx": np.ascontiguousarray(ctx_c[i * BPC:(i + 1) * BPC]),
            "w2rep": w2_rep,
        }
        for i in range(NCORES)
    ]
    return Gp, in_maps, counts


def _finish(res, Gp, counts):
    """Gather per-core outputs, host-side softmax denominator + divide."""
    S = Gp * J
    Tp = Gp * RPT
    num = np.concatenate(
        [np.asarray(res.results[i]["num"]) for i in range(NCORES)], axis=0
    ).astype(np.float32)
    pout = np.concatenate(
        [np.asarray(res.results[i]["pout"]) for i in range(NCORES)], axis=0
    ).astype(np.float32)
    # pout[b, p, g*J+j] holds p for context row t = g*RPT + p*J + j
    pv = pout.reshape(B, P, Gp, J).transpose(0, 2, 1, 3).reshape(B, Tp)
    den = np.array(
        [pv[b, :counts[b]].sum(dtype=np.float64) for b in range(B)],
        dtype=np.float32,
    )
    return num / den[:, None]


def kernel(query, context, mask, v_w):
    import time
    from concourse.bass_utils import run_bass_kernel_spmd

    Gp, in_maps, counts = _prepare(context, mask, v_w)
    nc = _get_program(Gp)
    last_err = None
    for attempt in range(3):
        try:
            res = run_bass_kernel_spmd(nc, in_maps, list(range(NCORES)))
            return _finish(res, Gp, counts)
        except Exception as e:  # transient axon/device hiccups
            last_err = e
            time.sleep(5)
    raise last_err
